# revision 4
# baseline (speedup 1.0000x reference)
"""Trainium2 Bass kernel for a 5x5 valid convolution over 96x96 images.

Reference computes x @ W.T where W is the [8464, 9216] conv-as-matmul
matrix (10 GFLOP dense).  We instead compute the convolution directly on
the tensor engine as 5 PSUM-accumulated banded matmuls (row-conv over the
image-row contraction, column shifts folded into the rhs access pattern):

    out[oi, b, oj] = sum_kj  B_kj.T @ X[:, b, oj+kj]
    B_kj[i, oi]    = K[i-oi, kj]   (banded Toeplitz, built on device)

Sharding: data-parallel over batch; each of the 8 cores convolves 8
images.  Raw Bass (no TileContext) — the static DAG is hand-scheduled to
avoid Tile's ~25us end-of-kernel drain/barrier epilogue.
"""

import sys

sys.path.insert(0, "/opt/trn_rl_repo")

import numpy as np

import bass_rust
import concourse.bass as bass
import concourse.mybir as mybir
from concourse.bass_utils import run_bass_kernel_spmd

# Problem geometry (hardcoded per the task contract).
BATCH = 64
IN = 96           # input image side
KD = 5            # conv kernel side
OD = IN - KD + 1  # output side = 92
ISIZE = IN * IN   # 9216
OSIZE = OD * OD   # 8464
NCORES = 8
BPC = BATCH // NCORES  # images per core = 8
HALF = BPC // 2        # images per PSUM accumulation group = 4
UL = 187               # per-kj stripe length in the padded tap vector u


def _ap(view, offset, dims):
    ap = view.copy()
    ap.offset = offset
    ap.ap = bass_rust.VecI64Pair(dims)
    return ap


def _build_program(warmup_mms=28):
    nc = bass.Bass()
    dt = mybir.dt.float32

    x_in = nc.declare_dram_parameter("x", [BPC, ISIZE], dt, isOutput=False)
    k_in = nc.declare_dram_parameter("k", [KD, KD], dt, isOutput=False)
    y_out = nc.declare_dram_parameter("y", [BPC, OSIZE], dt, isOutput=True)
    u_dram = nc.dram_tensor("u_scratch", [KD * UL], dt)

    with (
        nc.sbuf_tensor([1, KD * KD], dt) as k_sb,
        nc.sbuf_tensor([1, KD * UL], dt) as u_sb,
        nc.sbuf_tensor([IN, KD, OD], dt) as b_tmp,
        nc.sbuf_tensor([IN, KD, OD], dt) as b_sb,
        nc.sbuf_tensor([IN, BPC, IN], dt) as x_sb,
        nc.sbuf_tensor([OD, BPC, OD], dt) as out_sb,
        nc.sbuf_tensor([128, 128], dt) as junk_sb,
        nc.psum_tensor([OD, HALF, OD], dt) as ps0,
        nc.psum_tensor([OD, HALF, OD], dt) as ps1,
        nc.psum_tensor([128, 128], dt) as ps_junk,
        nc.semaphore() as sem_k,      # k -> k_sb
        nc.semaphore() as sem_x,      # x -> x_sb
        nc.semaphore() as sem_taps,   # taps scattered into u_sb
        nc.semaphore() as sem_uout,   # u_sb -> u_dram
        nc.semaphore() as sem_btmp,   # u_dram -> b_tmp
        nc.semaphore() as sem_brev,   # b_tmp reversed -> b_sb
        nc.semaphore() as sem_mm,     # psum group done
        nc.semaphore() as sem_copy,   # psum -> out_sb half done
        nc.semaphore() as sem_y,      # out_sb -> y
        nc.Block() as block,
    ):
        psums = [ps0, ps1]

        @block.sync
        def _(sync):
            sync.dma_start(
                out=k_sb[:], in_=k_in.rearrange("a b -> (a b)").unsqueeze(0)
            ).then_inc(sem_k, 16)
            sync.dma_start(
                out=x_sb[:],
                in_=_ap(x_in[:], 0, [[IN, IN], [ISIZE, BPC], [1, IN]]),
            ).then_inc(sem_x, 16)
            sync.wait_ge(sem_taps, 1)
            sync.dma_start(
                out=_ap(u_dram[:], 0, [[KD * UL, 1], [1, KD * UL]]), in_=u_sb[:]
            ).then_inc(sem_uout, 16)
            sync.wait_ge(sem_uout, 16)
            # B_tmp[p, kj, r] = u[kj*UL + p + r]  (= B[p, kj, 91-r])
            sync.dma_start(
                out=b_tmp[:],
                in_=_ap(u_dram[:], 0, [[1, IN], [UL, KD], [1, OD]]),
            ).then_inc(sem_btmp, 16)
            # y[b, oi*92+oj] <- out_sb[oi, b, oj], one half at a time
            for h in range(2):
                sync.wait_ge(sem_copy, h + 1)
                sync.dma_start(
                    out=_ap(
                        y_out[:],
                        h * HALF * OSIZE,
                        [[OD, OD], [OSIZE, HALF], [1, OD]],
                    ),
                    in_=out_sb[:, h * HALF : (h + 1) * HALF, :],
                ).then_inc(sem_y, 16)
            sync.wait_ge(sem_y, 32)

        @block.vector
        def _(vector):
            nc.vector.memset(u_sb[:], 0.0)
            vector.wait_ge(sem_k, 16)
            # u[kj*UL + 91 + t] = K[t, kj]
            nc.vector.tensor_copy(
                _ap(u_sb[:], OD - 1, [[KD * UL, 1], [UL, KD], [1, KD]]),
                _ap(k_sb[:], 0, [[KD * KD, 1], [1, KD], [KD, KD]]),
            ).then_inc(sem_taps, 1)
            vector.wait_ge(sem_btmp, 16)
            # reverse the oi axis: B[p, kj, oi] = B_tmp[p, kj, 91-oi]
            nc.vector.tensor_copy(
                b_sb[:],
                _ap(b_tmp[:], OD - 1, [[KD * OD, IN], [OD, KD], [-1, OD]]),
            ).then_inc(sem_brev, 1)
            for h in range(2):
                vector.wait_ge(sem_mm, h + 1)
                nc.vector.tensor_copy(
                    out_sb[:, h * HALF : (h + 1) * HALF, :], psums[h][:]
                ).then_inc(sem_copy, 1)

        @block.tensor
        def _(tensor):
            # HAM warmup: keep the PE busy while DMAs/B-build run so the
            # real matmuls execute at 2.4 GHz instead of 1.2 GHz.
            for _ in range(warmup_mms):
                nc.tensor.matmul(
                    ps_junk[:], junk_sb[:], junk_sb[:], start=True, stop=True
                )
            tensor.wait_ge(sem_brev, 1)
            tensor.wait_ge(sem_x, 16)
            for h in range(2):
                for kj in range(KD):
                    mm = nc.tensor.matmul(
                        psums[h][:],
                        b_sb[:, kj, :],
                        _ap(
                            x_sb[:],
                            h * HALF * IN + kj,
                            [[BPC * IN, IN], [IN, HALF], [1, OD]],
                        ),
                        start=(kj == 0),
                        stop=(kj == KD - 1),
                    )
                    if kj == KD - 1:
                        mm.then_inc(sem_mm, 1)

    return nc


_NC = None


def kernel(x: np.ndarray, kernel: np.ndarray) -> np.ndarray:
    global _NC
    if _NC is None:
        _NC = _build_program()

    x = np.ascontiguousarray(x, dtype=np.float32)
    k = np.ascontiguousarray(kernel, dtype=np.float32)
    in_maps = [
        {"x": x[c * BPC : (c + 1) * BPC], "k": k} for c in range(NCORES)
    ]
    res = run_bass_kernel_spmd(_NC, in_maps, list(range(NCORES)))
    return np.concatenate([res.results[c]["y"] for c in range(NCORES)], axis=0)
